# revision 3
# baseline (speedup 1.0000x reference)
"""Trainium2 Bass kernel for the stacked-Chebyshev locally-connected net.

Reference computation (B=256, k=6250, d*d=4096, O=10):
    x1 = z @ (mask*T1).T
    x2 = 2*(z @ (mask*T2).T)*x1 - T0
    x3 = 2*(z @ (mask*T3).T)*x2 - x1
    out = x3 @ C_w.T + C_b

The mask is a locally-connected conv pattern: 16x16 patch, stride 2, 25x25
positions, stacked 10x.  Rows that share the same patch-row index i have a
single contiguous, 128-aligned 1024-wide support in d — grouping by i cuts
the matmul contraction from 4096 to 1024 (4x fewer MACs than dense).

Sharding: 25 i-groups over 8 cores; every core gets 3 whole consecutive
groups plus 1/8 of group 24 (a 32-column "mini" unit whose d-window is the
same for all cores).  A full slot is 250 k-columns in 2 k-tiles of 128,
contracted over 8 K-chunks of a 10-chunk z.T window.

Performance structure (v2):
  - every DMA is a plain 2D copy of a host-preblocked array; ~14 issues
    total, split across the two HWDGE rings (sync, scalar) in exactly the
    order the PE consumes them, with the late/small loads on gpsimd's
    SWDGE ring so they never queue ahead of critical weights.
  - dummy matmuls on a memset tile run during the DMA preamble so the PE's
    HAM clock-gate is warm (2.4 GHz) when the real matmuls start.
  - all matmuls (layers + k->O projection) are fp16, so the compiler's
    fast-weight-load stays enabled everywhere and LDWEIGHTS hides behind
    the matmul stream.
  - the Chebyshev recurrence is split across the scalar (ACT) and vector
    (DVE) engines; per-core partial outputs are summed on the host.
"""

import numpy as np

import concourse.bass as bass
import concourse.mybir as mybir
import concourse.tile as tile
from concourse import bacc
from concourse.bass_utils import run_bass_kernel_spmd

F32 = mybir.dt.float32
F16 = mybir.dt.float16

B = 256          # batch
O = 10           # output classes
D2 = 4096        # d*d
N_CORES = 8
FULL_SLOTS = 3   # whole groups per core
SLOT_COLS = 256  # 2 k-tiles of 128 (125 real cols each)
MINI_COLS = 32   # k-columns of the shared group-24 mini unit (<=32 real)
WIN_CH = 10      # z.T window chunks per core (slot s uses chunks s..s+7)
G_SPLIT = 24     # the group split across all 8 cores
N_UNITS = 2 * FULL_SLOTS + 1
DUMMY_MMS = 10   # HAM warm-up matmuls during the DMA preamble

# columns of group G_SPLIT owned by each core (6x31 + 2x32 = 250)
_MINI_N = (31, 31, 31, 31, 31, 31, 32, 32)
_MINI_OFF = tuple(int(x) for x in np.cumsum((0,) + _MINI_N[:-1]))


def _group_cols(i):
    """k-column indices of patch-row-group i (order: stack-major, then j)."""
    return np.array(
        [s * 625 + i * 25 + j for s in range(10) for j in range(25)], dtype=np.int64
    )


def _build_nc():
    nc = bacc.Bacc(
        "TRN2", target_bir_lowering=False, debug=False, num_devices=N_CORES
    )
    # z.T window, chunk-blocked on the host: col block c <-> window chunk c
    zw = nc.dram_tensor("zw", [128, WIN_CH * B], F16, kind="ExternalInput").ap()
    # group-24 z.T slab, chunk-blocked
    zg = nc.dram_tensor("zg", [128, 8 * B], F16, kind="ExternalInput").ap()
    # full-slot weights; row block (3s+li) is the SBUF image for (slot, layer)
    wall = nc.dram_tensor("wall", [9 * 128, 8 * SLOT_COLS], F16,
                          kind="ExternalInput").ap()
    # mini-unit weights for all 3 layers: col = layer*256 + chunk*32 + n
    wm = nc.dram_tensor("wm", [128, 3 * 8 * MINI_COLS], F16,
                        kind="ExternalInput").ap()
    # negated T0 (additive bias on the scalar engine); col = unit index
    t0n = nc.dram_tensor("t0n", [128, 8], F32, kind="ExternalInput").ap()
    cwt = nc.dram_tensor("cwt", [128, N_UNITS * O], F16, kind="ExternalInput").ap()
    out = nc.dram_tensor("out", [O, B], F32, kind="ExternalOutput").ap()

    with tile.TileContext(nc) as tc:
        with (
            tc.tile_pool(name="zpool", bufs=1) as zpool,
            tc.tile_pool(name="cpool", bufs=1) as cpool,
            tc.tile_pool(name="wpool", bufs=1) as wpool,
            tc.tile_pool(name="xpool", bufs=6) as xpool,
            tc.tile_pool(name="tpool", bufs=4) as tpool,
            tc.tile_pool(name="ppool", bufs=6, space="PSUM") as ppool,
            tc.tile_pool(name="opool", bufs=1, space="PSUM") as opool,
            tc.tile_pool(name="dpool", bufs=1, space="PSUM") as dpool,
        ):
            # ---- dummy warm-up data (no DMA dependency) ----
            dz = zpool.tile([128, B], F16, tag="dz")
            nc.vector.memset(dz[:], 0.0)

            # ---- DMA issue plan ----
            # sync ring:   zw[0:4], w(s0,L1)a, w(s0,L1)b, w(s0,L3), w(s1,L2),
            #              w(s2,L1), w(s2,L3), out (at the end)
            # scalar ring: zw[4:10], w(s0,L2), w(s1,L1), w(s1,L3), w(s2,L2)
            # gpsimd ring: t0n, cwt, wm, zg
            zwa = zpool.tile([128, 4 * B], F16, tag="zwa")
            zwb = zpool.tile([128, 6 * B], F16, tag="zwb")
            nc.sync.dma_start(zwa[:], zw[:, 0:4 * B])
            nc.scalar.dma_start(zwb[:], zw[:, 4 * B:WIN_CH * B])

            def wslice(s, li):
                r = (3 * s + li) * 128
                return wall[r:r + 128, :]

            wt = {}
            # slot-0 layer-1 in two pieces so the first matmuls start early
            w00a = wpool.tile([128, 4 * SLOT_COLS], F16, tag="w00a")
            w00b = wpool.tile([128, 4 * SLOT_COLS], F16, tag="w00b")
            nc.sync.dma_start(w00a[:], wslice(0, 0)[:, 0:4 * SLOT_COLS])
            w01 = wpool.tile([128, 8 * SLOT_COLS], F16, tag="w01")
            wt[(1, 0)] = w01
            nc.scalar.dma_start(w01[:], wslice(0, 1))
            nc.sync.dma_start(w00b[:], wslice(0, 0)[:, 4 * SLOT_COLS:])

            for eng, s, li in ((nc.sync, 0, 2), (nc.scalar, 1, 0),
                               (nc.sync, 1, 1), (nc.scalar, 1, 2),
                               (nc.sync, 2, 0), (nc.scalar, 2, 1),
                               (nc.sync, 2, 2)):
                t = wpool.tile([128, 8 * SLOT_COLS], F16, tag=f"w{s}{li}")
                wt[(li, s)] = t
                eng.dma_start(t[:], wslice(s, li))

            t0_sb = cpool.tile([128, 8], F32, tag="t0")
            cw_sb = cpool.tile([128, N_UNITS * O], F16, tag="cw")
            wm_sb = cpool.tile([128, 3 * 8 * MINI_COLS], F16, tag="wm")
            zgt = cpool.tile([128, 8 * B], F16, tag="zg")
            nc.gpsimd.dma_start(t0_sb[:], t0n[:])
            nc.gpsimd.dma_start(cw_sb[:], cwt[:])
            nc.gpsimd.dma_start(wm_sb[:], wm[:])
            nc.gpsimd.dma_start(zgt[:], zg[:])

            # ---- HAM warm-up ----
            dps = dpool.tile([128, B], F32)
            for _ in range(DUMMY_MMS):
                nc.tensor.matmul(dps[:], dz[:, 0:128], dz[:], start=True,
                                 stop=True)

            def zch(c):
                if c < 4:
                    return zwa[:, c * B:(c + 1) * B]
                return zwb[:, (c - 4) * B:(c - 3) * B]

            psum_o = opool.tile([O, B], F32)
            n_proj = 0
            pending = []   # deferred projection matmuls (src, unit, rows)

            def flush_proj():
                nonlocal n_proj
                for src_t, unit, rows in pending:
                    n_proj += 1
                    nc.tensor.matmul(psum_o[:],
                                     cw_sb[0:rows, unit * O:(unit + 1) * O],
                                     src_t[:],
                                     start=(n_proj == 1),
                                     stop=(n_proj == N_UNITS))
                pending.clear()

            def epilogue(li, p, xs, unit, rows):
                """Per-layer recurrence for one k-tile unit on ACT + DVE."""
                if li == 0:
                    x1 = xpool.tile([rows, B], F32, tag="x1")
                    nc.scalar.copy(x1[:], p[:])
                    xs["x1"] = x1
                elif li == 1:
                    m2 = tpool.tile([rows, B], F32, tag="m2")
                    x2 = xpool.tile([rows, B], F32, tag="x2")
                    nc.vector.tensor_mul(m2[:], p[:], xs["x1"][:])
                    nc.scalar.add(x2[:], m2[:], t0_sb[0:rows, unit:unit + 1])
                    xs["x2"] = x2
                else:
                    u = tpool.tile([rows, B], F32, tag="u")
                    x3 = xpool.tile([rows, B], F16, tag="x3")
                    nc.vector.tensor_mul(u[:], p[:], xs["x2"][:])
                    nc.vector.tensor_sub(x3[:], u[:], xs["x1"][:])
                    pending.append((x3, unit, rows))

            def full_slot(s):
                units = [{}, {}]
                for li in range(3):
                    if s == 0 and li == 0:
                        wpieces = (w00a, w00b)
                    else:
                        wpieces = (wt[(li, s)],)
                    pa = ppool.tile([128, B], F32, tag="ps")
                    pb = ppool.tile([128, B], F32, tag="ps")
                    flush_proj()
                    for kc in range(8):
                        zc = zch(s + kc)
                        if len(wpieces) == 2:
                            w = wpieces[kc // 4]
                            col = (kc % 4) * SLOT_COLS
                        else:
                            w = wpieces[0]
                            col = kc * SLOT_COLS
                        nc.tensor.matmul(pa[:], w[:, col:col + 128], zc,
                                         start=(kc == 0), stop=(kc == 7))
                        nc.tensor.matmul(pb[:], w[:, col + 128:col + SLOT_COLS],
                                         zc, start=(kc == 0), stop=(kc == 7))
                    epilogue(li, pa, units[0], 2 * s + 0, 128)
                    epilogue(li, pb, units[1], 2 * s + 1, 128)

            full_slot(0)
            full_slot(1)
            full_slot(2)

            # mini unit last: its short [32, B] recurrence minimizes the tail
            mini = {}
            for li in range(3):
                p = ppool.tile([MINI_COLS, B], F32, tag="ps")
                flush_proj()
                for kc in range(8):
                    lhsT = wm_sb[:, li * 8 * MINI_COLS + kc * MINI_COLS:
                                 li * 8 * MINI_COLS + (kc + 1) * MINI_COLS]
                    nc.tensor.matmul(p[:], lhsT, zgt[:, kc * B:(kc + 1) * B],
                                     start=(kc == 0), stop=(kc == 7))
                epilogue(li, p, mini, 6, MINI_COLS)
            flush_proj()

            out_sb = cpool.tile([O, B], F32, tag="out")
            nc.vector.tensor_copy(out_sb[:], psum_o[:])
            nc.sync.dma_start(out[:], out_sb[:])

    nc.compile()
    return nc


_NC = None


def _get_nc():
    global _NC
    if _NC is None:
        _NC = _build_nc()
    return _NC


def _prepare_in_maps(z, T1, T2, T3, T0, C_w, mask):
    z = np.ascontiguousarray(np.asarray(z, dtype=np.float32).reshape(B, D2))
    T1 = np.asarray(T1, dtype=np.float32)
    T2 = np.asarray(T2, dtype=np.float32)
    T3 = np.asarray(T3, dtype=np.float32)
    T0 = np.asarray(T0, dtype=np.float32)
    C_w = np.asarray(C_w, dtype=np.float32)
    mask = np.asarray(mask, dtype=np.float32)

    zT = np.ascontiguousarray(z.T)                   # [4096, 256]
    Ts = (T1, T2, T3)
    scales = (1.0, 2.0, 2.0)
    g24_cols = _group_cols(G_SPLIT)
    g24_win = np.arange(128 * G_SPLIT, 128 * G_SPLIT + 1024)

    # group-24 z slab, chunk-blocked: [128, 8*B]
    zg_blk = (zT[128 * G_SPLIT:128 * G_SPLIT + 1024]
              .reshape(8, 128, B).transpose(1, 0, 2)
              .reshape(128, 8 * B)).astype(np.float16)

    in_maps = []
    for c in range(N_CORES):
        i0 = 3 * c
        zw_blk = (zT[128 * i0:128 * i0 + WIN_CH * 128]
                  .reshape(WIN_CH, 128, B).transpose(1, 0, 2)
                  .reshape(128, WIN_CH * B)).astype(np.float16)
        m = {"zw": np.ascontiguousarray(zw_blk),
             "zg": np.ascontiguousarray(zg_blk)}

        # full-slot weights: [9*128, 8*256], row block (3s+li)
        wts = np.zeros((FULL_SLOTS, 3, 1024, SLOT_COLS), np.float32)
        for s in range(FULL_SLOTS):
            g = i0 + s
            cols = _group_cols(g)
            ix = np.ix_(cols, np.arange(128 * g, 128 * g + 1024))
            mk = mask[ix]
            for li, (T, sc) in enumerate(zip(Ts, scales)):
                AT = (sc * T[ix] * mk).T          # [1024, 250]
                wts[s, li, :, 0:125] = AT[:, 0:125]
                wts[s, li, :, 128:253] = AT[:, 125:250]
        m["wall"] = np.ascontiguousarray(
            wts.reshape(FULL_SLOTS, 3, 8, 128, SLOT_COLS)
            .transpose(0, 1, 3, 2, 4)
            .reshape(9 * 128, 8 * SLOT_COLS)).astype(np.float16)

        # mini unit
        nmini = _MINI_N[c]
        mcols = g24_cols[_MINI_OFF[c]:_MINI_OFF[c] + nmini]
        wmh = np.zeros((128, 3, 8, MINI_COLS), np.float32)
        for li, (T, sc) in enumerate(zip(Ts, scales)):
            A = (sc * T[np.ix_(mcols, g24_win)] * mask[np.ix_(mcols, g24_win)]).T
            wmh[:, li, :, 0:nmini] = A.reshape(8, 128, nmini).transpose(1, 0, 2)
        m["wm"] = np.ascontiguousarray(
            wmh.reshape(128, 3 * 8 * MINI_COLS)).astype(np.float16)

        # t0 / C_w per unit: units 0..5 = full slots (2s+kt), 6 = mini
        t0nh = np.zeros((128, 8), np.float32)
        cwth = np.zeros((128, N_UNITS * O), np.float32)
        for s in range(FULL_SLOTS):
            cols = _group_cols(i0 + s)
            t0nh[0:125, 2 * s] = -T0[cols[0:125]]
            t0nh[0:125, 2 * s + 1] = -T0[cols[125:250]]
            cwth[0:125, (2 * s) * O:(2 * s + 1) * O] = C_w[:, cols[0:125]].T
            cwth[0:125, (2 * s + 1) * O:(2 * s + 2) * O] = C_w[:, cols[125:250]].T
        t0nh[0:nmini, 6] = -T0[mcols]
        cwth[0:nmini, 6 * O:7 * O] = C_w[:, mcols].T
        m["t0n"] = t0nh
        m["cwt"] = cwth.astype(np.float16)
        in_maps.append(m)
    return in_maps


def kernel(z, T1, T2, T3, T0, C_w, C_b, mask):
    nc = _get_nc()
    in_maps = _prepare_in_maps(z, T1, T2, T3, T0, C_w, mask)
    res = run_bass_kernel_spmd(nc, in_maps, core_ids=list(range(N_CORES)))
    total = np.zeros((O, B), np.float32)
    for c in range(N_CORES):
        total += res.results[c]["out"]
    C_b = np.asarray(C_b, dtype=np.float32)
    return (total.T + C_b).astype(np.float32)
